# revision 24
# baseline (speedup 1.0000x reference)
"""Trainium2 Bass kernel for nn_AttentionBlock (B=2, C=256, D=8, H=32, W=32).

reference math:
    xf = x.reshape(B, C, N)                        # N = 8192
    q = wq @ xf + bq                               # (B, 32, N)
    k = wk @ xf + bk                               # (B, 32, N)
    v = wv @ xf + bv                               # (B, 256, N)
    attn = softmax(q^T k, axis=-1)                 # (B, N, N)
    out = attn @ v^T                               # (B, N, C) buffer
    result = gamma * out.reshape(B, C, d, h, w) + x

Sharding (8 cores): core i -> batch b = i//4, query-chunk c = i%4 of 2048
rows.  No collectives.

v4 design (fp8 DoubleRow + 4-quadrant S + split exp):
  - projections: fp8e4 weights (prescaled x64 on host, descaled in the
    bias pass) x fp8 xf, DoubleRow K=256 matmuls.
  - scores: S^T = k^T q as bf16 K=32 matmuls, 4 key tiles per superstep
    packed on all 4 row-quadrants via tile_position (keeps the PE array
    dense so the HAM clock gate stays at K=8/8 = 2.4 GHz).
  - softmax exp (|S| < 4.5, no max subtraction): the superstep's 4 PSUM
    banks are split in half: banks 0-1 -> DVE (Schraudolph bit-trick
    round(a*S+b) as int8 bitcast fp8e4), banks 2-3 -> ScalarE (ACTIVATE
    Exp -> fp8).  Both engines run concurrently inside one superstep, so
    a single S-buffer set suffices and the PE never waits on exp.
  - attn @ v: fp8 DoubleRow, 2 key-pair matmuls x 4 query tiles per
    superstep.  vT is 272-padded (16B-aligned pair stride) and carries a
    WS=64.0 column so PSUM accumulates WS*rowsum next to out (vT holds
    WS-prescaled v; the scales cancel in the epilogue divide).
  - V-projection tail (m=32..63) is interleaved into sc=0's supersteps
    as PE filler, so the attention pipe-fill has no long PE gaps.
  - epilogue: rec = 1/psum[:,256]; out = (psum[:,0:256]*rec) + xres'
    in one DVE scalar_tensor_tensor; xres' = x-slice + gamma*bv (host).
PSUM: 4 banks S (single set, half-split exp) + 4 out accumulators.
"""

import numpy as np

B, C, Dd, Hh, Ww = 2, 256, 8, 32, 32
N = Dd * Hh * Ww          # 8192
CQK = C // 8              # 32
NCORES = 8
QCHUNK = N // 4           # 2048 query rows per core
P = 128

WS = 64.0                 # host-side weight prescale (fp8 range)
A_F8 = 11.5416            # 2**3 / ln2  (fp8e4 schraudolph)
B_F8 = 55.7248            # 8 * (7 - 0.0344)
VPAD = 272                # 16B-aligned vT tile pitch (257 used)


def build_graph(n=N, nq=QCHUNK):
    import concourse.tile as tile
    from concourse import bacc, mybir
    from concourse.bass import ds, ts

    f32 = mybir.dt.float32
    bf16 = mybir.dt.bfloat16
    fp8 = mybir.dt.float8e4
    i8 = mybir.dt.int8
    AF = mybir.ActivationFunctionType
    ALU = mybir.AluOpType
    DR = mybir.MatmulPerfMode.DoubleRow

    n_t = n // 512            # 16 K-proj tiles
    m_tiles = n // P          # 64 key tiles (V proj)
    n_sc = nq // 512          # 4 query chunks
    n_ss2 = n // 256          # 32 supersteps (2 key tiles each) per sc

    nc = bacc.Bacc()
    xf8_d = nc.declare_dram_parameter("xf8", [C, n], fp8, isOutput=False)
    xq8_d = nc.declare_dram_parameter("xq8", [C, nq], fp8, isOutput=False)
    xres_d = nc.declare_dram_parameter("xres", [nq, C], f32, isOutput=False)
    wq8_d = nc.declare_dram_parameter("wq8", [C, CQK], fp8, isOutput=False)
    wk8_d = nc.declare_dram_parameter("wk8", [C, CQK], fp8, isOutput=False)
    wv8_d = nc.declare_dram_parameter("wv8", [C, C], fp8, isOutput=False)
    bq_d = nc.declare_dram_parameter("bq", [CQK, 1], f32, isOutput=False)
    bk_d = nc.declare_dram_parameter("bk", [CQK, 1], f32, isOutput=False)
    out_d = nc.declare_dram_parameter("out", [nq, C], f32, isOutput=True)

    with tile.TileContext(nc) as tc:
        with tc.tile_pool(name="singles", bufs=1) as singles, \
             tc.tile_pool(name="ostage", bufs=3) as ostage, \
             tc.tile_pool(name="small", bufs=4) as small, \
             tc.tile_pool(name="ptp", bufs=12) as ptp:

            # dummy matmuls: keep the PE busy from t=0 so the HAM clock
            # gate releases to 8/8 before the real projections arrive
            # (junk memset first in the gpsimd queue, ahead of DMA issues)
            junk = singles.tile([P, 640], bf16)
            nc.gpsimd.memset(junk, 0.25)
            with tc.tile_pool(name="warmp", bufs=2, space="PSUM") as wp:
                for r in range(40):
                    wps = wp.tile([P, 512], f32, tag="w", name=f"wps{r}")
                    nc.tensor.matmul(wps, lhsT=junk[:, 0:P],
                                     rhs=junk[:, P:P + 512],
                                     start=True, stop=True)

            # ---- weights / biases ----------------------------------------
            wq8_s = singles.tile([P, 2, CQK], fp8)
            wk8_s = singles.tile([P, 2, CQK], fp8)
            wv8_s = singles.tile([P, 2, C], fp8)
            for d, sb in ((wq8_d, wq8_s), (wk8_d, wk8_s), (wv8_d, wv8_s)):
                nc.gpsimd.dma_start(out=sb[:], in_=d[:].rearrange(
                    "(co p) m -> p co m", p=P))
            bq_s = singles.tile([P, 1], f32)
            bk_s = singles.tile([P, 1], f32)
            nc.gpsimd.dma_start(out=bq_s[0:CQK, :], in_=bq_d[:])
            nc.gpsimd.dma_start(out=bk_s[0:CQK, :], in_=bk_d[:])

            # warm the Exp activation table early (SBUF->SBUF dummy)
            warm = small.tile([P, 1], bf16, tag="warm", name="warm")
            nc.scalar.activation(warm[0:CQK, :], bq_s[0:CQK, :], AF.Exp)

            # ---- x loads --------------------------------------------------
            xf8_s = singles.tile([P, 2, n], fp8)
            xfr = xf8_d[:].rearrange("(co p) m -> p co m", p=P)
            for t in range(4):
                nc.sync.dma_start(out=xf8_s[:, :, ts(t, n // 4)],
                                  in_=xfr[:, :, ts(t, n // 4)])
            xq8_s = singles.tile([P, 2, nq], fp8)
            nc.sync.dma_start(out=xq8_s, in_=xq8_d[:].rearrange(
                "(co p) m -> p co m", p=P))
            xres_s = singles.tile([P, nq // P, C], f32)

            # ---- projections ---------------------------------------------
            # k_bd: per 128-key tile a block-diagonal [128, 128] lhsT --
            # rows 32a:32a+32 hold k[:, sub-block a] (dense PE stream, no
            # tile_position).  Built from k_stage by 4 scatter DMAs.
            k_stage = singles.tile([P, n_t, 512], bf16)
            k_bd = singles.tile([P, n // P, P], bf16)
            nc.gpsimd.memset(k_bd, 0.0)
            q_rep = singles.tile([P, n_sc, 512], bf16)
            vT8 = singles.tile([P, m_tiles, VPAD], fp8)
            nc.vector.memset(vT8[:, :, C:C + 1], WS)

            with tc.tile_pool(name="pp", bufs=4, space="PSUM") as pp:
                # q first (unblocks sc=0); bias+descale alternates engines
                for t in range(n_sc):
                    ps_q = pp.tile([P, 512], f32, tag="pskq", name=f"ps_q{t}")
                    nc.tensor.matmul(ps_q[0:CQK, :], lhsT=wq8_s,
                                     rhs=xq8_s[:, :, ts(t, 512)],
                                     start=True, stop=True, perf_mode=DR)
                    if t % 2 == 0:
                        nc.scalar.activation(q_rep[0:CQK, t, :], ps_q[0:CQK, :],
                                             AF.Identity, bias=bq_s[0:CQK, :],
                                             scale=1.0 / WS)
                    else:
                        nc.vector.tensor_scalar(
                            out=q_rep[0:CQK, t, :], in0=ps_q[0:CQK, :],
                            scalar1=1.0 / WS, scalar2=bq_s[0:CQK, :],
                            op0=ALU.mult, op1=ALU.add)
                # k tiles; bias+descale alternates ScalarE / DVE
                for t in range(n_t):
                    ps_k = pp.tile([P, 512], f32, tag="pskq", name=f"ps_k{t}")
                    nc.tensor.matmul(ps_k[0:CQK, :], lhsT=wk8_s,
                                     rhs=xf8_s[:, :, ts(t, 512)],
                                     start=True, stop=True, perf_mode=DR)
                    if t % 2 == 0:
                        nc.scalar.activation(k_stage[0:CQK, t, :], ps_k[0:CQK, :],
                                             AF.Identity, bias=bk_s[0:CQK, :],
                                             scale=1.0 / WS)
                    else:
                        nc.vector.tensor_scalar(
                            out=k_stage[0:CQK, t, :], in0=ps_k[0:CQK, :],
                            scalar1=1.0 / WS, scalar2=bk_s[0:CQK, :],
                            op0=ALU.mult, op1=ALU.add)
                for j in range(1, 4):
                    nc.sync.dma_start(out=q_rep[ds(32 * j, 32), :, :],
                                      in_=q_rep[0:32, :, :])
                # scatter k into the block diagonals: k_bd[32a+ch, kt, 32a+kk]
                ksr = k_stage[0:CQK, :, :].rearrange(
                    "p t (f a kk) -> p (t f) a kk", a=4, kk=32)
                for a in range(4):
                    nc.gpsimd.dma_start(
                        out=k_bd[ds(32 * a, 32), :, ds(32 * a, 32)],
                        in_=ksr[:, :, a, :])
                # v: vT8[m, c] = fp8(WS * gamma * (wv @ xf)); descale in
                # epi.  Two m-tiles share a PSUM bank -> one 512-col cast.
                for mp in range(m_tiles // 2):
                    ps_v = pp.tile([P, 2, C], f32, tag="psv", name=f"ps_v{mp}")
                    for h in range(2):
                        nc.tensor.matmul(ps_v[:, h, :],
                                         lhsT=xf8_s[:, :, ts(2 * mp + h, P)],
                                         rhs=wv8_s, start=True, stop=True,
                                         perf_mode=DR)
                    dst = vT8[:, ds(2 * mp, 2), 0:C]
                    srcv = ps_v[:]
                    if mp % 2 == 0:
                        nc.vector.tensor_copy(dst, srcv)
                    else:
                        nc.scalar.activation(dst, srcv, AF.Copy)
                # xres arrives late (first use ~epilogue of sc0)
                xrr = xres_d[:].rearrange("(t p) c -> p t c", p=P)
                for t in range(2):
                    nc.scalar.dma_start(
                        out=xres_s[:, ts(t, nq // P // 2), :],
                        in_=xrr[:, ts(t, nq // P // 2), :])

            # ---- attention ------------------------------------------------
            outr = out_d[:].rearrange("(t p) c -> p t c", p=P)
            with tc.tile_pool(name="stp", bufs=2, space="PSUM") as stp, \
                 tc.tile_pool(name="op", bufs=1, space="PSUM") as op:
                for sc in range(n_sc):
                    out_ps = [op.tile([P, VPAD], f32, tag=f"ops{qt}",
                                      name=f"out_ps{sc}_{qt}")
                              for qt in range(4)]
                    pipe = []
                    for ss in range(n_ss2):
                        s_ps = stp.tile([P, 2, 512], f32, tag="s",
                                        name=f"s{sc}_{ss}")
                        pt = ptp.tile([P, 2, 512], fp8, tag="pt",
                                      name=f"pt{sc}_{ss}")
                        for j2 in range(2):
                            t = 2 * ss + j2
                            nc.tensor.matmul(
                                s_ps[:, j2, :],
                                lhsT=k_bd[:, t, :],
                                rhs=q_rep[:, sc, :],
                                start=True, stop=True)
                        # one 1024-col exp per superstep; ~9/16 ScalarE
                        # (exact exp->fp8), rest DVE (schraudolph int8)
                        g = sc * n_ss2 + ss
                        if (g * 9) % 16 < 9:
                            nc.scalar.activation(
                                pt[:].rearrange("p t f -> p (t f)"),
                                s_ps[:].rearrange("p t f -> p (t f)"),
                                AF.Exp)
                        else:
                            nc.vector.tensor_scalar(
                                out=pt[:].rearrange(
                                    "p t f -> p (t f)").bitcast(i8),
                                in0=s_ps[:].rearrange("p t f -> p (t f)"),
                                scalar1=A_F8, scalar2=B_F8,
                                op0=ALU.mult, op1=ALU.add)
                        pipe.append((ss, pt))
                        if len(pipe) > 2:
                            pss, ppt = pipe.pop(0)
                            for qt in range(4):
                                nc.tensor.matmul(
                                    out_ps[qt][:, 0:257],
                                    lhsT=ppt[:, :, ts(qt, P)],
                                    rhs=vT8[:, ds(2 * pss, 2), 0:257],
                                    start=(pss == 0), stop=False,
                                    perf_mode=DR)
                    for pss, ppt in pipe:
                        for qt in range(4):
                            nc.tensor.matmul(
                                out_ps[qt][:, 0:257],
                                lhsT=ppt[:, :, ts(qt, P)],
                                rhs=vT8[:, ds(2 * pss, 2), 0:257],
                                start=(pss == 0),
                                stop=(pss == n_ss2 - 1), perf_mode=DR)
                    # epilogue: out = psum[:, :C] / (WS*rowsum) * WS + xres'
                    for qt in range(4):
                        rec = small.tile([P, 1], f32, tag="rec",
                                         name=f"rec{sc}_{qt}")
                        nc.vector.reciprocal(rec, out_ps[qt][:, 256:257])
                        ot = ostage.tile([P, C], f32, tag="ot",
                                         name=f"ot{sc}_{qt}")
                        nc.vector.scalar_tensor_tensor(
                            out=ot, in0=out_ps[qt][:, 0:C], scalar=rec,
                            in1=xres_s[:, 4 * sc + qt, :],
                            op0=ALU.mult, op1=ALU.add)
                        nc.gpsimd.dma_start(out=outr[:, 4 * sc + qt, :], in_=ot)
    nc.compile()
    return nc


_nc_cache = {}


def _get_graph(n=N, nq=QCHUNK):
    key = (n, nq)
    if key not in _nc_cache:
        _nc_cache[key] = build_graph(n, nq)
    return _nc_cache[key]


def _make_in_maps(x, wq, bq, wk, bk, wv, bv, gamma, n=N, nq=QCHUNK):
    import ml_dtypes
    f8 = ml_dtypes.float8_e4m3fn
    xf = np.ascontiguousarray(x.reshape(B, C, n)).astype(np.float32)
    xf8 = xf.astype(f8)
    g = float(np.asarray(gamma).reshape(-1)[0])
    wq8 = np.ascontiguousarray(np.asarray(wq, dtype=np.float32).T * WS).astype(f8)
    wk8 = np.ascontiguousarray(np.asarray(wk, dtype=np.float32).T * WS).astype(f8)
    wv8 = np.ascontiguousarray(
        np.asarray(wv, dtype=np.float32).T * (WS * g)).astype(f8)
    bq2 = np.asarray(bq, dtype=np.float32).reshape(CQK, 1)
    bk2 = np.asarray(bk, dtype=np.float32).reshape(CQK, 1)
    gbv = (g * np.asarray(bv, dtype=np.float32))[None, :]
    nchunks = n // nq
    in_maps = []
    for i in range(NCORES):
        b, c = divmod(i, nchunks)
        n0 = c * nq
        xres = xf[b].reshape(-1)[n0 * C:(n0 + nq) * C].reshape(nq, C) + gbv
        in_maps.append({
            "xf8": xf8[b],
            "xq8": np.ascontiguousarray(xf8[b][:, n0:n0 + nq]),
            "xres": np.ascontiguousarray(xres, dtype=np.float32),
            "wq8": wq8, "wk8": wk8, "wv8": wv8,
            "bq": bq2, "bk": bk2,
        })
    return in_maps


def _assemble(results, n=N, nq=QCHUNK):
    nchunks = n // nq
    outs = []
    for b in range(B):
        buf = np.concatenate(
            [results[b * nchunks + c]["out"] for c in range(nchunks)], axis=0)
        outs.append(buf.reshape(C, Dd, Hh, Ww))
    return np.stack(outs).astype(np.float32)


def kernel(x, wq, bq, wk, bk, wv, bv, gamma):
    from concourse.bass_utils import run_bass_kernel_spmd
    nc = _get_graph()
    in_maps = _make_in_maps(x, wq, bq, wk, bk, wv, bv, gamma)
    res = run_bass_kernel_spmd(nc, in_maps, core_ids=list(range(NCORES)))
    return _assemble(res.results)
